# revision 15
# baseline (speedup 1.0000x reference)
"""Trainium2 Bass kernel for CustomMHA (B=4, L=2048, D=1024, H=16, DK=64), fp32.

Sharding: 8 cores = 4 batches x 2 head-groups (8 heads each).
Each core computes, for its (batch b, head-group g):
  qkv = x_b @ Win_slice.T + b_slice       (f16 matmuls, fp32 accum)
  per head: S^T = k q^T * scale; A = exp(S^T); z^T = [v|1]^T-weighted sums
  partial_out = z_hat @ proj_slice.T      ([2048, 1024] fp32, no proj_b)
Host sums the two head-group partials per batch and adds proj_b.

Layout notes (per core):
  xT    [1024, 2048] f16  (x_b transposed; k-tiles are matmul lhsT/rhs)
  wqkT  [1024, 1024] f16  (rows [Wq_g; Wk_g] transposed)
  wvT   [1024, 512]  f16
  bqk   [128, 8] f32      (bias for q,k features; [partition, feature-tile])
  bv    [1, 512] f16
  projT [512, 1024] f16   (proj_w[:, g-cols] transposed)
  out   [2048, 1024] f32

The kernel is self-contained: shapes/sharding hardcoded, no file reads.
"""

import numpy as np
from contextlib import ExitStack

import concourse.bass as bass
import concourse.mybir as mybir
import concourse.tile as tile
from concourse import bacc
from concourse.bass_utils import run_bass_kernel_spmd

# Problem constants
B, L, D, H = 4, 2048, 1024, 16
DK = D // H                     # 64
SCALE = 1.0 / float(np.sqrt(DK))

# Per-core constants
P = 128
T = L                           # tokens per core (one batch)
NH = H // 2                     # 8 heads per core
DH = NH * DK                    # 512
NKT = D // P                    # 8 k-tiles over model dim
NTT = T // P                    # 16 token tiles
NQC = T // 512                  # 4 query chunks of 512
F16 = mybir.dt.float16
F32 = mybir.dt.float32

_EXTRA_EXP = False

# PSUM budget: tag "s" (S^T pair tiles) 2 bufs x 2 banks = 4, tag "z"
# (attention z accumulators) 2 bufs x 1 = 2, tag "m" (qkv/proj) 2 x 1 = 2.


_PSUM_SPLIT = True


def build_program(reps=1, extra_exp=False, psum_split=True):
    global _EXTRA_EXP, _PSUM_SPLIT
    _EXTRA_EXP = extra_exp
    _PSUM_SPLIT = psum_split
    nc = bacc.Bacc("TRN2", target_bir_lowering=False, debug=False,
                   enable_asserts=False, num_devices=8)

    xT = nc.dram_tensor("xT", [D, T], F16, kind="ExternalInput").ap()
    wqkT = nc.dram_tensor("wqkT", [D, 2 * DH], F16, kind="ExternalInput").ap()
    wvT = nc.dram_tensor("wvT", [D, DH], F16, kind="ExternalInput").ap()
    bqk = nc.dram_tensor("bqk", [P, NKT], F32, kind="ExternalInput").ap()
    bv = nc.dram_tensor("bv", [1, DH], F16, kind="ExternalInput").ap()
    projT = nc.dram_tensor("projT", [DH, D], F16, kind="ExternalInput").ap()
    out = nc.dram_tensor("out", [T, D], F32, kind="ExternalOutput").ap()

    with tile.TileContext(nc) as tc:
        with ExitStack() as ctx:
            _emit(nc, tc, ctx, xT, wqkT, wvT, bqk, bv, projT, out, reps)
    nc.compile()
    return nc


def _emit(nc, tc, ctx, xT, wqkT, wvT, bqk, bv, projT, out, reps=1):
    pers = ctx.enter_context(tc.tile_pool(name="pers", bufs=1))
    apool = ctx.enter_context(tc.tile_pool(name="apool", bufs=20))
    rpool = ctx.enter_context(tc.tile_pool(name="rpool", bufs=2))
    opool = ctx.enter_context(tc.tile_pool(name="opool", bufs=2))
    pspool = ctx.enter_context(tc.tile_pool(name="pspool", bufs=1, space="PSUM"))

    # ---- constant / weight / input loads ----
    bqk_sb = pers.tile([P, NKT], F32, name="bqk_sb")
    nc.sync.dma_start(bqk_sb[:], bqk[:])
    bv_sb = pers.tile([1, DH], F16, name="bv_sb")
    nc.sync.dma_start(bv_sb[:], bv[:])
    bvB = pers.tile([P, DH], F16, name="bvB")
    nc.gpsimd.partition_broadcast(bvB[:], bv_sb[:])

    wqk_sb = []
    x_sb = []
    wv_sb = []
    for ki in range(NKT):
        w = pers.tile([P, 2 * DH], F16, name=f"wqk_sb{ki}")
        nc.sync.dma_start(w[:], wqkT[ki * P:(ki + 1) * P, :])
        wqk_sb.append(w)
        xx = pers.tile([P, T], F16, name=f"x_sb{ki}")
        nc.sync.dma_start(xx[:], xT[ki * P:(ki + 1) * P, :])
        x_sb.append(xx)
    for ki in range(NKT):
        w = pers.tile([P, DH], F16, name=f"wv_sb{ki}")
        nc.sync.dma_start(w[:], wvT[ki * P:(ki + 1) * P, :])
        wv_sb.append(w)
    projT_sb = []
    for ki in range(DH // P):
        w = pers.tile([P, D], F16, name=f"projT_sb{ki}")
        nc.sync.dma_start(w[:], projT[ki * P:(ki + 1) * P, :])
        projT_sb.append(w)

    qk_sb = [pers.tile([P, T], F16, name=f"qk_sb{mi}") for mi in range(NKT)]
    vbuf = [pers.tile([P, NH, DK + 1], F16, name=f"vbuf{ti}") for ti in range(NTT)]
    zt_sb = [pers.tile([P, T], F16, name=f"zt_sb{ki}") for ki in range(DH // P)]

    for ti in range(NTT):
        nc.vector.memset(vbuf[ti][:, :, DK:DK + 1], 1.0)

    for rep in range(reps):
        _emit_compute(nc, tc, pers, apool, rpool, opool, pspool,
                      wqk_sb, x_sb, wv_sb, projT_sb, qk_sb, vbuf, zt_sb,
                      bqk_sb, bvB, out, rep)


def _emit_compute(nc, tc, pers, apool, rpool, opool, pspool,
                  wqk_sb, x_sb, wv_sb, projT_sb, qk_sb, vbuf, zt_sb,
                  bqk_sb, bvB, out, rep=0):

    def emit_qk(mi):
        for tcn in range(NQC):
            ps = pspool.tile([P, 512], F32, tag="m" if _PSUM_SPLIT else "z",
                             bufs=2, name=f"ps_qk{mi}_{tcn}")
            for ki in range(NKT):
                nc.tensor.matmul(
                    ps[:],
                    wqk_sb[ki][:, mi * P:(mi + 1) * P],
                    x_sb[ki][:, tcn * 512:(tcn + 1) * 512],
                    start=(ki == 0), stop=(ki == NKT - 1),
                )
            nc.vector.tensor_scalar_add(
                qk_sb[mi][:, tcn * 512:(tcn + 1) * 512], ps[:],
                bqk_sb[:, mi:mi + 1],
            )

    def emit_v(ti):
        ps = pspool.tile([P, 512], F32, tag="m" if _PSUM_SPLIT else "z",
                         bufs=2, name=f"ps_v{ti}")
        for ki in range(NKT):
            nc.tensor.matmul(
                ps[:],
                x_sb[ki][:, ti * P:(ti + 1) * P],
                wv_sb[ki][:],
                start=(ki == 0), stop=(ki == NKT - 1),
            )
        nc.vector.tensor_add(
            vbuf[ti][:, :, 0:DK],
            ps.rearrange("p (h j) -> p h j", h=NH),
            bvB.rearrange("p (h j) -> p h j", h=NH),
        )

    # Head pairs are row-packed on the PE: even head uses array rows 0-63,
    # odd head rows 64-127; their S matmuls run concurrently. One psum tile
    # [128, 2, 512] holds both heads' S^T chunk for a kt tile; one exp
    # instruction covers both.
    def emit_attn(qc, pr):
        qcs = slice(qc * 512, (qc + 1) * 512)
        lhe, lho = 2 * pr, 2 * pr + 1
        qtile = qk_sb[pr]
        ktile = qk_sb[4 + pr]
        q_e = qtile[0:64, qcs]
        q_o = qtile[64:128, qcs]

        a_tiles = []
        for kt in range(NTT):
            kts = slice(kt * P, (kt + 1) * P)
            ps = pspool.tile([P, 2, 512], F32, tag="s",
                             bufs=2 if _PSUM_SPLIT else 3,
                             name=f"ps_s{qc}_{pr}_{kt}")
            nc.tensor.matmul(ps[:, 0, :], ktile[0:64, kts], q_e,
                             start=True, stop=True, tile_position=(0, 0))
            nc.tensor.matmul(ps[:, 1, :], ktile[64:128, kts], q_o,
                             start=True, stop=True, tile_position=(64, 0))
            a = apool.tile([P, 2, 512], F16, tag="A",
                           name=f"a_{qc}_{pr}_{kt}")
            nc.scalar.activation(a, ps[:], mybir.ActivationFunctionType.Exp,
                                 scale=SCALE)
            if _EXTRA_EXP:
                a2 = apool.tile([P, 2, 512], F16, tag="Ascr", bufs=2,
                                name=f"a2_{qc}_{pr}_{kt}")
                nc.scalar.activation(a2, ps[:],
                                     mybir.ActivationFunctionType.Exp,
                                     scale=SCALE)
            a_tiles.append(a)

        # z accumulations for the two heads interleave kt-wise so each A
        # tile is released as soon as both heads consumed it.
        psz_e = pspool.tile([P, 512], F32, tag="z", bufs=2,
                            name=f"ps_z{qc}_{lhe}")
        psz_o = pspool.tile([P, 512], F32, tag="z", bufs=2,
                            name=f"ps_z{qc}_{lho}")
        for kt in range(NTT):
            for psz, idx, lh in ((psz_e, 0, lhe), (psz_o, 1, lho)):
                nc.tensor.matmul(
                    psz[0:DK + 1, :],
                    vbuf[kt][:, lh, :],
                    a_tiles[kt][:, idx, :],
                    start=(kt == 0), stop=(kt == NTT - 1),
                )
        for psz, idx, lh in ((psz_e, 0, lhe), (psz_o, 1, lho)):
            row = 64 * idx
            recip = rpool.tile([1, 512], F32, tag="rc", name=f"rc_{qc}_{lh}")
            nc.vector.reciprocal(recip[:], psz[DK:DK + 1, :])
            recipB = rpool.tile([64, 512], F32, tag="rb", name=f"rb_{qc}_{lh}")
            nc.gpsimd.partition_broadcast(recipB[:], recip[:])
            nc.vector.tensor_mul(
                zt_sb[pr][row:row + 64, qcs],
                psz[0:DK, :],
                recipB[:],
            )

    def emit_proj(ti):
        ot = opool.tile([P, D], F32, tag="ot", name=f"ot{ti}")
        for ocn in range(2):
            ps = pspool.tile([P, 512], F32, tag="m" if _PSUM_SPLIT else "z",
                             bufs=2, name=f"ps_o{ti}_{ocn}")
            for ki in range(DH // P):
                nc.tensor.matmul(
                    ps[:],
                    zt_sb[ki][:, ti * P:(ti + 1) * P],
                    projT_sb[ki][:, ocn * 512:(ocn + 1) * 512],
                    start=(ki == 0), stop=(ki == DH // P - 1),
                )
            nc.vector.tensor_copy(ot[:, ocn * 512:(ocn + 1) * 512], ps[:])
        nc.sync.dma_start(out[ti * P:(ti + 1) * P, :], ot[:])

    # Emission order: q,k feature tiles (head pairs early), v, then qc-outer
    # attention with the qc's token projections right after each qc completes.
    for mi in (0, 4, 1, 5, 2, 6, 3, 7):
        emit_qk(mi)
    for ti in range(NTT):
        emit_v(ti)
    for qc in range(NQC):
        for pr in range(NH // 2):
            emit_attn(qc, pr)
        for ti in range(qc * NTT // NQC, (qc + 1) * NTT // NQC):
            emit_proj(ti)


_NC_CACHE = None


def _get_program():
    global _NC_CACHE
    if _NC_CACHE is None:
        _NC_CACHE = build_program()
    return _NC_CACHE


def shard_inputs(x, Win_w, Win_b, proj_w, proj_b):
    """Build the 8 per-core input maps (host-side numpy)."""
    in_maps = []
    for c in range(8):
        b, g = divmod(c, 2)
        qs = slice(g * DH, (g + 1) * DH)
        ks = slice(D + g * DH, D + (g + 1) * DH)
        vs = slice(2 * D + g * DH, 2 * D + (g + 1) * DH)
        wqk = np.concatenate([Win_w[qs], Win_w[ks]], axis=0)      # [1024, 1024]
        bqk_v = np.concatenate([Win_b[qs], Win_b[ks]])            # [1024]
        in_maps.append({
            "xT": np.ascontiguousarray(x[b].T.astype(np.float16)),
            "wqkT": np.ascontiguousarray(wqk.T.astype(np.float16)),
            "wvT": np.ascontiguousarray(Win_w[vs].T.astype(np.float16)),
            "bqk": np.ascontiguousarray(
                bqk_v.reshape(NKT, P).T.astype(np.float32)),
            "bv": Win_b[vs].astype(np.float16).reshape(1, DH),
            "projT": np.ascontiguousarray(
                proj_w[:, g * DH:(g + 1) * DH].T.astype(np.float16)),
        })
    return in_maps


def combine_outputs(results, proj_b):
    out = np.empty((B, L, D), dtype=np.float32)
    pb = proj_b.astype(np.float32)
    for b in range(B):
        out[b] = results[2 * b]["out"] + results[2 * b + 1]["out"] + pb
    return out


def kernel(x, Win_w, Win_b, proj_w, proj_b):
    x = np.asarray(x, dtype=np.float32)
    Win_w = np.asarray(Win_w, dtype=np.float32)
    Win_b = np.asarray(Win_b, dtype=np.float32)
    proj_w = np.asarray(proj_w, dtype=np.float32)
    proj_b = np.asarray(proj_b, dtype=np.float32)

    nc = _get_program()
    in_maps = shard_inputs(x, Win_w, Win_b, proj_w, proj_b)
    res = run_bass_kernel_spmd(nc, in_maps, core_ids=list(range(8)))
    return combine_outputs(res.results, proj_b)


# revision 16
# speedup vs baseline: 1.0374x; 1.0374x over previous
"""Trainium2 Bass kernel for CustomMHA (B=4, L=2048, D=1024, H=16, DK=64), fp32.

Sharding: 8 cores = 4 batches x 2 head-groups (8 heads each).
Each core computes, for its (batch b, head-group g):
  qkv = x_b @ Win_slice.T + b_slice       (f16 matmuls, fp32 accum)
  per head: S^T = k q^T * scale; A = exp(S^T); z^T = [v|1]^T-weighted sums
  partial_out = z_hat @ proj_slice.T      ([2048, 1024] fp32, no proj_b)
Host sums the two head-group partials per batch and adds proj_b.

Layout notes (per core):
  xT    [1024, 2048] f16  (x_b transposed; k-tiles are matmul lhsT/rhs)
  wqkT  [1024, 1024] f16  (rows [Wq_g; Wk_g] transposed)
  wvT   [1024, 512]  f16
  bqk   [128, 8] f32      (bias for q,k features; [partition, feature-tile])
  bv    [1, 512] f16
  projT [512, 1024] f16   (proj_w[:, g-cols] transposed)
  out   [2048, 1024] f32

The kernel is self-contained: shapes/sharding hardcoded, no file reads.
"""

import numpy as np
from contextlib import ExitStack

import concourse.bass as bass
import concourse.mybir as mybir
import concourse.tile as tile
from concourse import bacc
from concourse.bass_utils import run_bass_kernel_spmd

# Problem constants
B, L, D, H = 4, 2048, 1024, 16
DK = D // H                     # 64
SCALE = 1.0 / float(np.sqrt(DK))

# Per-core constants
P = 128
T = L                           # tokens per core (one batch)
NH = H // 2                     # 8 heads per core
DH = NH * DK                    # 512
NKT = D // P                    # 8 k-tiles over model dim
NTT = T // P                    # 16 token tiles
NQC = T // 512                  # 4 query chunks of 512
F16 = mybir.dt.float16
F32 = mybir.dt.float32

_EXTRA_EXP = False
_HALF_EXP = False

# PSUM budget: tag "s" (S^T pair tiles) 2 bufs x 2 banks = 4, tag "z"
# (attention z accumulators) 2 bufs x 1 = 2, tag "m" (qkv/proj) 2 x 1 = 2.


_PSUM_SPLIT = True


def build_program(reps=1, extra_exp=False, psum_split=True, half_exp=False):
    global _EXTRA_EXP, _PSUM_SPLIT, _HALF_EXP
    _EXTRA_EXP = extra_exp
    _PSUM_SPLIT = psum_split
    _HALF_EXP = half_exp
    nc = bacc.Bacc("TRN2", target_bir_lowering=False, debug=False,
                   enable_asserts=False, num_devices=8)

    xT = nc.dram_tensor("xT", [D, T], F16, kind="ExternalInput").ap()
    wqkT = nc.dram_tensor("wqkT", [D, 2 * DH], F16, kind="ExternalInput").ap()
    wvT = nc.dram_tensor("wvT", [D, DH], F16, kind="ExternalInput").ap()
    bqk = nc.dram_tensor("bqk", [P, NKT], F32, kind="ExternalInput").ap()
    bv = nc.dram_tensor("bv", [1, DH], F16, kind="ExternalInput").ap()
    projT = nc.dram_tensor("projT", [DH, D], F16, kind="ExternalInput").ap()
    out = nc.dram_tensor("out", [T, D], F32, kind="ExternalOutput").ap()

    with tile.TileContext(nc) as tc:
        with ExitStack() as ctx:
            _emit(nc, tc, ctx, xT, wqkT, wvT, bqk, bv, projT, out, reps)
    nc.compile()
    return nc


def _emit(nc, tc, ctx, xT, wqkT, wvT, bqk, bv, projT, out, reps=1):
    pers = ctx.enter_context(tc.tile_pool(name="pers", bufs=1))
    apool = ctx.enter_context(tc.tile_pool(name="apool", bufs=20))
    rpool = ctx.enter_context(tc.tile_pool(name="rpool", bufs=2))
    opool = ctx.enter_context(tc.tile_pool(name="opool", bufs=2))
    pspool = ctx.enter_context(tc.tile_pool(name="pspool", bufs=1, space="PSUM"))

    # ---- constant / weight / input loads ----
    bqk_sb = pers.tile([P, NKT], F32, name="bqk_sb")
    nc.sync.dma_start(bqk_sb[:], bqk[:])
    bv_sb = pers.tile([1, DH], F16, name="bv_sb")
    nc.sync.dma_start(bv_sb[:], bv[:])
    bvB = pers.tile([P, DH], F16, name="bvB")
    nc.gpsimd.partition_broadcast(bvB[:], bv_sb[:])

    wqk_sb = []
    x_sb = []
    wv_sb = []
    for ki in range(NKT):
        w = pers.tile([P, 2 * DH], F16, name=f"wqk_sb{ki}")
        nc.sync.dma_start(w[:], wqkT[ki * P:(ki + 1) * P, :])
        wqk_sb.append(w)
        xx = pers.tile([P, T], F16, name=f"x_sb{ki}")
        nc.sync.dma_start(xx[:], xT[ki * P:(ki + 1) * P, :])
        x_sb.append(xx)
    for ki in range(NKT):
        w = pers.tile([P, DH], F16, name=f"wv_sb{ki}")
        nc.sync.dma_start(w[:], wvT[ki * P:(ki + 1) * P, :])
        wv_sb.append(w)
    projT_sb = []
    for ki in range(DH // P):
        w = pers.tile([P, D], F16, name=f"projT_sb{ki}")
        nc.sync.dma_start(w[:], projT[ki * P:(ki + 1) * P, :])
        projT_sb.append(w)

    qk_sb = [pers.tile([P, T], F16, name=f"qk_sb{mi}") for mi in range(NKT)]
    vbuf = [pers.tile([P, NH, DK + 1], F16, name=f"vbuf{ti}") for ti in range(NTT)]
    zt_sb = [pers.tile([P, T], F16, name=f"zt_sb{ki}") for ki in range(DH // P)]

    for ti in range(NTT):
        nc.vector.memset(vbuf[ti][:, :, DK:DK + 1], 1.0)

    for rep in range(reps):
        _emit_compute(nc, tc, pers, apool, rpool, opool, pspool,
                      wqk_sb, x_sb, wv_sb, projT_sb, qk_sb, vbuf, zt_sb,
                      bqk_sb, bvB, out, rep)


def _emit_compute(nc, tc, pers, apool, rpool, opool, pspool,
                  wqk_sb, x_sb, wv_sb, projT_sb, qk_sb, vbuf, zt_sb,
                  bqk_sb, bvB, out, rep=0):

    def emit_qk(mi):
        for tcn in range(NQC):
            ps = pspool.tile([P, 512], F32, tag="m" if _PSUM_SPLIT else "z",
                             bufs=2, name=f"ps_qk{mi}_{tcn}")
            for ki in range(NKT):
                nc.tensor.matmul(
                    ps[:],
                    wqk_sb[ki][:, mi * P:(mi + 1) * P],
                    x_sb[ki][:, tcn * 512:(tcn + 1) * 512],
                    start=(ki == 0), stop=(ki == NKT - 1),
                )
            nc.vector.tensor_scalar_add(
                qk_sb[mi][:, tcn * 512:(tcn + 1) * 512], ps[:],
                bqk_sb[:, mi:mi + 1],
            )

    def emit_v(ti):
        ps = pspool.tile([P, 512], F32, tag="m" if _PSUM_SPLIT else "z",
                         bufs=2, name=f"ps_v{ti}")
        for ki in range(NKT):
            nc.tensor.matmul(
                ps[:],
                x_sb[ki][:, ti * P:(ti + 1) * P],
                wv_sb[ki][:],
                start=(ki == 0), stop=(ki == NKT - 1),
            )
        nc.vector.tensor_add(
            vbuf[ti][:, :, 0:DK],
            ps.rearrange("p (h j) -> p h j", h=NH),
            bvB.rearrange("p (h j) -> p h j", h=NH),
        )

    # Head pairs are row-packed on the PE: even head uses array rows 0-63,
    # odd head rows 64-127; their S matmuls run concurrently. One psum tile
    # [128, 2, 512] holds both heads' S^T chunk for a kt tile; one exp
    # instruction covers both.
    def emit_attn(qc, pr):
        qcs = slice(qc * 512, (qc + 1) * 512)
        lhe, lho = 2 * pr, 2 * pr + 1
        qtile = qk_sb[pr]
        ktile = qk_sb[4 + pr]
        q_e = qtile[0:64, qcs]
        q_o = qtile[64:128, qcs]

        a_tiles = []
        for kt in range(NTT):
            kts = slice(kt * P, (kt + 1) * P)
            ps = pspool.tile([P, 2, 512], F32, tag="s",
                             bufs=2 if _PSUM_SPLIT else 3,
                             name=f"ps_s{qc}_{pr}_{kt}")
            nc.tensor.matmul(ps[:, 0, :], ktile[0:64, kts], q_e,
                             start=True, stop=True, tile_position=(0, 0))
            nc.tensor.matmul(ps[:, 1, :], ktile[64:128, kts], q_o,
                             start=True, stop=True, tile_position=(64, 0))
            a = apool.tile([P, 2, 512], F16, tag="A",
                           name=f"a_{qc}_{pr}_{kt}")
            if _HALF_EXP:
                # timing diagnostic: half the ACT work, identical PE/DVE work
                nc.scalar.activation(a[:, 0, :], ps[:, 0, :],
                                     mybir.ActivationFunctionType.Exp,
                                     scale=SCALE)
            else:
                nc.scalar.activation(a, ps[:],
                                     mybir.ActivationFunctionType.Exp,
                                     scale=SCALE)
            if _EXTRA_EXP:
                a2 = apool.tile([P, 2, 512], F16, tag="Ascr", bufs=2,
                                name=f"a2_{qc}_{pr}_{kt}")
                nc.scalar.activation(a2, ps[:],
                                     mybir.ActivationFunctionType.Exp,
                                     scale=SCALE)
            a_tiles.append(a)

        # z accumulations for the two heads interleave kt-wise so each A
        # tile is released as soon as both heads consumed it.
        psz_e = pspool.tile([P, 512], F32, tag="z", bufs=2,
                            name=f"ps_z{qc}_{lhe}")
        psz_o = pspool.tile([P, 512], F32, tag="z", bufs=2,
                            name=f"ps_z{qc}_{lho}")
        for kt in range(NTT):
            for psz, idx, lh in ((psz_e, 0, lhe), (psz_o, 1, lho)):
                nc.tensor.matmul(
                    psz[0:DK + 1, :],
                    vbuf[kt][:, lh, :],
                    a_tiles[kt][:, 0 if _HALF_EXP else idx, :],
                    start=(kt == 0), stop=(kt == NTT - 1),
                )
        for psz, idx, lh in ((psz_e, 0, lhe), (psz_o, 1, lho)):
            row = 64 * idx
            recip = rpool.tile([1, 512], F32, tag="rc", name=f"rc_{qc}_{lh}")
            nc.vector.reciprocal(recip[:], psz[DK:DK + 1, :])
            recipB = rpool.tile([64, 512], F32, tag="rb", name=f"rb_{qc}_{lh}")
            nc.gpsimd.partition_broadcast(recipB[:], recip[:])
            nc.vector.tensor_mul(
                zt_sb[pr][row:row + 64, qcs],
                psz[0:DK, :],
                recipB[:],
            )

    def emit_proj(ti):
        ot = opool.tile([P, D], F32, tag="ot", name=f"ot{ti}")
        for ocn in range(2):
            ps = pspool.tile([P, 512], F32, tag="m" if _PSUM_SPLIT else "z",
                             bufs=2, name=f"ps_o{ti}_{ocn}")
            for ki in range(DH // P):
                nc.tensor.matmul(
                    ps[:],
                    zt_sb[ki][:, ti * P:(ti + 1) * P],
                    projT_sb[ki][:, ocn * 512:(ocn + 1) * 512],
                    start=(ki == 0), stop=(ki == DH // P - 1),
                )
            nc.vector.tensor_copy(ot[:, ocn * 512:(ocn + 1) * 512], ps[:])
        nc.sync.dma_start(out[ti * P:(ti + 1) * P, :], ot[:])

    # Emission order: q,k feature tiles (head pairs early), v, then qc-outer
    # attention with the qc's token projections right after each qc completes.
    for mi in (0, 4, 1, 5, 2, 6, 3, 7):
        emit_qk(mi)
    for ti in range(NTT):
        emit_v(ti)
    for qc in range(NQC):
        for pr in range(NH // 2):
            emit_attn(qc, pr)
        for ti in range(qc * NTT // NQC, (qc + 1) * NTT // NQC):
            emit_proj(ti)


_NC_CACHE = None


def _get_program():
    global _NC_CACHE
    if _NC_CACHE is None:
        _NC_CACHE = build_program()
    return _NC_CACHE


def shard_inputs(x, Win_w, Win_b, proj_w, proj_b):
    """Build the 8 per-core input maps (host-side numpy)."""
    in_maps = []
    for c in range(8):
        b, g = divmod(c, 2)
        qs = slice(g * DH, (g + 1) * DH)
        ks = slice(D + g * DH, D + (g + 1) * DH)
        vs = slice(2 * D + g * DH, 2 * D + (g + 1) * DH)
        wqk = np.concatenate([Win_w[qs], Win_w[ks]], axis=0)      # [1024, 1024]
        bqk_v = np.concatenate([Win_b[qs], Win_b[ks]])            # [1024]
        in_maps.append({
            "xT": np.ascontiguousarray(x[b].T.astype(np.float16)),
            "wqkT": np.ascontiguousarray(wqk.T.astype(np.float16)),
            "wvT": np.ascontiguousarray(Win_w[vs].T.astype(np.float16)),
            "bqk": np.ascontiguousarray(
                bqk_v.reshape(NKT, P).T.astype(np.float32)),
            "bv": Win_b[vs].astype(np.float16).reshape(1, DH),
            "projT": np.ascontiguousarray(
                proj_w[:, g * DH:(g + 1) * DH].T.astype(np.float16)),
        })
    return in_maps


def combine_outputs(results, proj_b):
    out = np.empty((B, L, D), dtype=np.float32)
    pb = proj_b.astype(np.float32)
    for b in range(B):
        out[b] = results[2 * b]["out"] + results[2 * b + 1]["out"] + pb
    return out


def kernel(x, Win_w, Win_b, proj_w, proj_b):
    x = np.asarray(x, dtype=np.float32)
    Win_w = np.asarray(Win_w, dtype=np.float32)
    Win_b = np.asarray(Win_b, dtype=np.float32)
    proj_w = np.asarray(proj_w, dtype=np.float32)
    proj_b = np.asarray(proj_b, dtype=np.float32)

    nc = _get_program()
    in_maps = shard_inputs(x, Win_w, Win_b, proj_w, proj_b)
    res = run_bass_kernel_spmd(nc, in_maps, core_ids=list(range(8)))
    return combine_outputs(res.results, proj_b)


# revision 18
# speedup vs baseline: 1.1100x; 1.0700x over previous
"""Trainium2 Bass kernel for CustomMHA (B=4, L=2048, D=1024, H=16, DK=64), fp32.

Sharding: 8 cores = 4 batches x 2 head-groups (8 heads each).
Each core computes, for its (batch b, head-group g):
  qkv = x_b @ Win_slice.T + b_slice       (f16 matmuls, fp32 accum)
  per head: S^T = k q^T * scale; A = exp(S^T); z^T = [v|1]^T-weighted sums
  partial_out = z_hat @ proj_slice.T      ([2048, 1024] fp32, no proj_b)
Host sums the two head-group partials per batch and adds proj_b.

Layout notes (per core):
  xT    [1024, 2048] f16  (x_b transposed; k-tiles are matmul lhsT/rhs)
  wqkT  [1024, 1024] f16  (rows [Wq_g; Wk_g] transposed)
  wvT   [1024, 512]  f16
  bqk   [128, 8] f32      (bias for q,k features; [partition, feature-tile])
  bv    [1, 512] f16
  projT [512, 1024] f16   (proj_w[:, g-cols] transposed)
  out   [2048, 1024] f32

The kernel is self-contained: shapes/sharding hardcoded, no file reads.
"""

import numpy as np
from contextlib import ExitStack

import concourse.bass as bass
import concourse.mybir as mybir
import concourse.tile as tile
from concourse import bacc
from concourse.bass_utils import run_bass_kernel_spmd

# Problem constants
B, L, D, H = 4, 2048, 1024, 16
DK = D // H                     # 64
SCALE = 1.0 / float(np.sqrt(DK))

# Per-core constants
P = 128
T = L                           # tokens per core (one batch)
NH = H // 2                     # 8 heads per core
DH = NH * DK                    # 512
NKT = D // P                    # 8 k-tiles over model dim
NTT = T // P                    # 16 token tiles
NQC = T // 512                  # 4 query chunks of 512
F16 = mybir.dt.float16
F32 = mybir.dt.float32

_EXTRA_EXP = False
_HALF_EXP = False

# PSUM budget: tag "s" (S^T pair tiles) 2 bufs x 2 banks = 4, tag "z"
# (attention z accumulators) 2 bufs x 1 = 2, tag "m" (qkv/proj) 2 x 1 = 2.


_PSUM_SPLIT = True


def build_program(reps=1, extra_exp=False, psum_split=True, half_exp=False):
    global _EXTRA_EXP, _PSUM_SPLIT, _HALF_EXP
    _EXTRA_EXP = extra_exp
    _PSUM_SPLIT = psum_split
    _HALF_EXP = half_exp
    nc = bacc.Bacc("TRN2", target_bir_lowering=False, debug=False,
                   enable_asserts=False, num_devices=8)

    xT = nc.dram_tensor("xT", [D, T], F16, kind="ExternalInput").ap()
    wqkT = nc.dram_tensor("wqkT", [D, 2 * DH], F16, kind="ExternalInput").ap()
    wvT = nc.dram_tensor("wvT", [D, DH], F16, kind="ExternalInput").ap()
    bqk = nc.dram_tensor("bqk", [P, NKT], F32, kind="ExternalInput").ap()
    bv = nc.dram_tensor("bv", [1, DH], F16, kind="ExternalInput").ap()
    projT = nc.dram_tensor("projT", [DH, D], F16, kind="ExternalInput").ap()
    out = nc.dram_tensor("out", [T, D], F32, kind="ExternalOutput").ap()

    with tile.TileContext(nc) as tc:
        with ExitStack() as ctx:
            _emit(nc, tc, ctx, xT, wqkT, wvT, bqk, bv, projT, out, reps)
    nc.compile()
    return nc


def _emit(nc, tc, ctx, xT, wqkT, wvT, bqk, bv, projT, out, reps=1):
    pers = ctx.enter_context(tc.tile_pool(name="pers", bufs=1))
    apool = ctx.enter_context(tc.tile_pool(name="apool", bufs=20))
    rpool = ctx.enter_context(tc.tile_pool(name="rpool", bufs=2))
    opool = ctx.enter_context(tc.tile_pool(name="opool", bufs=2))
    pspool = ctx.enter_context(tc.tile_pool(name="pspool", bufs=1, space="PSUM"))

    # ---- constant / weight / input loads ----
    bqk_sb = pers.tile([P, NKT], F32, name="bqk_sb")
    nc.sync.dma_start(bqk_sb[:], bqk[:])
    bv_sb = pers.tile([1, DH], F16, name="bv_sb")
    nc.sync.dma_start(bv_sb[:], bv[:])
    bvB = pers.tile([P, DH], F16, name="bvB")
    nc.gpsimd.partition_broadcast(bvB[:], bv_sb[:])

    wqk_sb = []
    x_sb = []
    wv_sb = []
    for ki in range(NKT):
        w = pers.tile([P, 2 * DH], F16, name=f"wqk_sb{ki}")
        nc.sync.dma_start(w[:], wqkT[ki * P:(ki + 1) * P, :])
        wqk_sb.append(w)
        xx = pers.tile([P, T], F16, name=f"x_sb{ki}")
        nc.sync.dma_start(xx[:], xT[ki * P:(ki + 1) * P, :])
        x_sb.append(xx)
    for ki in range(NKT):
        w = pers.tile([P, DH], F16, name=f"wv_sb{ki}")
        nc.sync.dma_start(w[:], wvT[ki * P:(ki + 1) * P, :])
        wv_sb.append(w)
    projT_sb = []
    for ki in range(DH // P):
        w = pers.tile([P, D], F16, name=f"projT_sb{ki}")
        nc.sync.dma_start(w[:], projT[ki * P:(ki + 1) * P, :])
        projT_sb.append(w)

    qk_sb = [pers.tile([P, T], F16, name=f"qk_sb{mi}") for mi in range(NKT)]
    vbuf = [pers.tile([P, NH, DK + 1], F16, name=f"vbuf{ti}") for ti in range(NTT)]
    zt_sb = [pers.tile([P, T], F16, name=f"zt_sb{ki}") for ki in range(DH // P)]

    for ti in range(NTT):
        nc.vector.memset(vbuf[ti][:, :, DK:DK + 1], 1.0)

    for rep in range(reps):
        _emit_compute(nc, tc, pers, apool, rpool, opool, pspool,
                      wqk_sb, x_sb, wv_sb, projT_sb, qk_sb, vbuf, zt_sb,
                      bqk_sb, bvB, out, rep)


def _emit_compute(nc, tc, pers, apool, rpool, opool, pspool,
                  wqk_sb, x_sb, wv_sb, projT_sb, qk_sb, vbuf, zt_sb,
                  bqk_sb, bvB, out, rep=0):

    def emit_qk(mi):
        for tcn in range(NQC):
            ps = pspool.tile([P, 512], F32, tag="m" if _PSUM_SPLIT else "z",
                             bufs=2, name=f"ps_qk{mi}_{tcn}")
            for ki in range(NKT):
                nc.tensor.matmul(
                    ps[:],
                    wqk_sb[ki][:, mi * P:(mi + 1) * P],
                    x_sb[ki][:, tcn * 512:(tcn + 1) * 512],
                    start=(ki == 0), stop=(ki == NKT - 1),
                )
            nc.vector.tensor_scalar_add(
                qk_sb[mi][:, tcn * 512:(tcn + 1) * 512], ps[:],
                bqk_sb[:, mi:mi + 1],
            )

    def emit_v(ti):
        ps = pspool.tile([P, 512], F32, tag="m" if _PSUM_SPLIT else "z",
                         bufs=2, name=f"ps_v{ti}")
        for ki in range(NKT):
            nc.tensor.matmul(
                ps[:],
                x_sb[ki][:, ti * P:(ti + 1) * P],
                wv_sb[ki][:],
                start=(ki == 0), stop=(ki == NKT - 1),
            )
        nc.vector.tensor_add(
            vbuf[ti][:, :, 0:DK],
            ps.rearrange("p (h j) -> p h j", h=NH),
            bvB.rearrange("p (h j) -> p h j", h=NH),
        )

    # Head pairs are row-packed on the PE: even head uses array rows 0-63,
    # odd head rows 64-127; their S matmuls run concurrently. One psum tile
    # [128, 2, 512] holds both heads' S^T chunk for a kt tile; one exp
    # instruction covers both.
    def emit_attn(qc, pr):
        qcs = slice(qc * 512, (qc + 1) * 512)
        lhe, lho = 2 * pr, 2 * pr + 1
        qtile = qk_sb[pr]
        ktile = qk_sb[4 + pr]
        q_e = qtile[0:64, qcs]
        q_o = qtile[64:128, qcs]

        a_tiles = []
        for kt in range(NTT):
            kts = slice(kt * P, (kt + 1) * P)
            ps = pspool.tile([P, 2, 512], F32, tag="s",
                             bufs=2 if _PSUM_SPLIT else 3,
                             name=f"ps_s{qc}_{pr}_{kt}")
            nc.tensor.matmul(ps[:, 0, :], ktile[0:64, kts], q_e,
                             start=True, stop=True, tile_position=(0, 0))
            nc.tensor.matmul(ps[:, 1, :], ktile[64:128, kts], q_o,
                             start=True, stop=True, tile_position=(64, 0))
            a = apool.tile([P, 2, 512], F16, tag="A",
                           name=f"a_{qc}_{pr}_{kt}")
            if _HALF_EXP:
                # timing diagnostic: half the ACT work, identical PE/DVE work
                nc.scalar.activation(a[:, 0, :], ps[:, 0, :],
                                     mybir.ActivationFunctionType.Exp,
                                     scale=SCALE)
            else:
                nc.scalar.activation(a, ps[:],
                                     mybir.ActivationFunctionType.Exp,
                                     scale=SCALE)
            if _EXTRA_EXP:
                a2 = apool.tile([P, 2, 512], F16, tag="Ascr", bufs=2,
                                name=f"a2_{qc}_{pr}_{kt}")
                nc.scalar.activation(a2, ps[:],
                                     mybir.ActivationFunctionType.Exp,
                                     scale=SCALE)
            a_tiles.append(a)

        # z accumulations for the two heads interleave kt-wise so each A
        # tile is released as soon as both heads consumed it.
        psz_e = pspool.tile([P, 512], F32, tag="z", bufs=2,
                            name=f"ps_z{qc}_{lhe}")
        psz_o = pspool.tile([P, 512], F32, tag="z", bufs=2,
                            name=f"ps_z{qc}_{lho}")
        for kt in range(NTT):
            for psz, idx, lh in ((psz_e, 0, lhe), (psz_o, 1, lho)):
                nc.tensor.matmul(
                    psz[0:DK + 1, :],
                    vbuf[kt][:, lh, :],
                    a_tiles[kt][:, 0 if _HALF_EXP else idx, :],
                    start=(kt == 0), stop=(kt == NTT - 1),
                )
        for psz, idx, lh in ((psz_e, 0, lhe), (psz_o, 1, lho)):
            row = 64 * idx
            recip = rpool.tile([1, 512], F32, tag="rc", name=f"rc_{qc}_{lh}")
            nc.vector.reciprocal(recip[:], psz[DK:DK + 1, :])
            recipB = rpool.tile([64, 512], F32, tag="rb", name=f"rb_{qc}_{lh}")
            nc.gpsimd.partition_broadcast(recipB[:], recip[:])
            nc.vector.tensor_mul(
                zt_sb[pr][row:row + 64, qcs],
                psz[0:DK, :],
                recipB[:],
            )

    def emit_proj(ti):
        ot = opool.tile([P, D], F32, tag="ot", name=f"ot{ti}")
        for ocn in range(2):
            ps = pspool.tile([P, 512], F32, tag="m" if _PSUM_SPLIT else "z",
                             bufs=2, name=f"ps_o{ti}_{ocn}")
            for ki in range(DH // P):
                nc.tensor.matmul(
                    ps[:],
                    zt_sb[ki][:, ti * P:(ti + 1) * P],
                    projT_sb[ki][:, ocn * 512:(ocn + 1) * 512],
                    start=(ki == 0), stop=(ki == DH // P - 1),
                )
            nc.vector.tensor_copy(ot[:, ocn * 512:(ocn + 1) * 512], ps[:])
        nc.sync.dma_start(out[ti * P:(ti + 1) * P, :], ot[:])

    # Emission order: q,k feature tiles (head pairs early), v, then qc-outer
    # attention with the qc's token projections right after each qc completes.
    for mi in (0, 4, 1, 5, 2, 6, 3, 7):
        emit_qk(mi)
    for ti in range(NTT):
        emit_v(ti)
    for qc in range(NQC):
        for pr in range(NH // 2):
            emit_attn(qc, pr)
        for ti in range(qc * NTT // NQC, (qc + 1) * NTT // NQC):
            emit_proj(ti)


_NC_CACHE = None


def _get_program():
    global _NC_CACHE
    if _NC_CACHE is None:
        _NC_CACHE = build_program()
    return _NC_CACHE


def shard_inputs(x, Win_w, Win_b, proj_w, proj_b):
    """Build the 8 per-core input maps (host-side numpy)."""
    in_maps = []
    for c in range(8):
        b, g = divmod(c, 2)
        qs = slice(g * DH, (g + 1) * DH)
        ks = slice(D + g * DH, D + (g + 1) * DH)
        vs = slice(2 * D + g * DH, 2 * D + (g + 1) * DH)
        wqk = np.concatenate([Win_w[qs], Win_w[ks]], axis=0)      # [1024, 1024]
        bqk_v = np.concatenate([Win_b[qs], Win_b[ks]])            # [1024]
        in_maps.append({
            "xT": np.ascontiguousarray(x[b].T.astype(np.float16)),
            "wqkT": np.ascontiguousarray(wqk.T.astype(np.float16)),
            "wvT": np.ascontiguousarray(Win_w[vs].T.astype(np.float16)),
            "bqk": np.ascontiguousarray(
                bqk_v.reshape(NKT, P).T.astype(np.float32)),
            "bv": Win_b[vs].astype(np.float16).reshape(1, DH),
            "projT": np.ascontiguousarray(
                proj_w[:, g * DH:(g + 1) * DH].T.astype(np.float16)),
        })
    return in_maps


def combine_outputs(results, proj_b):
    out = np.empty((B, L, D), dtype=np.float32)
    pb = proj_b.astype(np.float32)
    for b in range(B):
        out[b] = results[2 * b]["out"] + results[2 * b + 1]["out"] + pb
    return out


def kernel(x, Win_w, Win_b, proj_w, proj_b):
    x = np.asarray(x, dtype=np.float32)
    Win_w = np.asarray(Win_w, dtype=np.float32)
    Win_b = np.asarray(Win_b, dtype=np.float32)
    proj_w = np.asarray(proj_w, dtype=np.float32)
    proj_b = np.asarray(proj_b, dtype=np.float32)

    nc = _get_program()
    in_maps = shard_inputs(x, Win_w, Win_b, proj_w, proj_b)
    res = run_bass_kernel_spmd(nc, in_maps, core_ids=list(range(8)))
    return combine_outputs(res.results, proj_b)


# revision 19
# speedup vs baseline: 1.2003x; 1.0814x over previous
"""Trainium2 Bass kernel for CustomMHA (B=4, L=2048, D=1024, H=16, DK=64), fp32.

Sharding: 8 cores = 4 batches x 2 head-groups (8 heads each).
Each core computes, for its (batch b, head-group g):
  qkv = x_b @ Win_slice.T + b_slice       (f16 matmuls, fp32 accum)
  per head: S^T = k q^T * scale; A = exp(S^T); z^T = [v|1]^T-weighted sums
  partial_out = z_hat @ proj_slice.T      ([2048, 1024] fp32, no proj_b)
Host sums the two head-group partials per batch and adds proj_b.

Layout notes (per core):
  xT    [1024, 2048] f16  (x_b transposed; k-tiles are matmul lhsT/rhs)
  wqkT  [1024, 1024] f16  (rows [Wq_g; Wk_g] transposed)
  wvT   [1024, 512]  f16
  bqk   [128, 8] f32      (bias for q,k features; [partition, feature-tile])
  bv    [1, 512] f16
  projT [512, 1024] f16   (proj_w[:, g-cols] transposed)
  out   [2048, 1024] f32

The kernel is self-contained: shapes/sharding hardcoded, no file reads.
"""

import numpy as np
from contextlib import ExitStack

import concourse.bass as bass
import concourse.mybir as mybir
import concourse.tile as tile
from concourse import bacc
from concourse.bass_utils import run_bass_kernel_spmd

# Problem constants
B, L, D, H = 4, 2048, 1024, 16
DK = D // H                     # 64
SCALE = 1.0 / float(np.sqrt(DK))

# Per-core constants
P = 128
T = L                           # tokens per core (one batch)
NH = H // 2                     # 8 heads per core
DH = NH * DK                    # 512
NKT = D // P                    # 8 k-tiles over model dim
NTT = T // P                    # 16 token tiles
NQC = T // 512                  # 4 query chunks of 512
F16 = mybir.dt.float16
F32 = mybir.dt.float32

_EXTRA_EXP = False
_HALF_EXP = False

# PSUM budget: tag "s" (S^T pair tiles) 2 bufs x 2 banks = 4, tag "z"
# (attention z accumulators) 2 bufs x 1 = 2, tag "m" (qkv/proj) 2 x 1 = 2.


_PSUM_SPLIT = True


def build_program(reps=1, extra_exp=False, psum_split=True, half_exp=False):
    global _EXTRA_EXP, _PSUM_SPLIT, _HALF_EXP
    _EXTRA_EXP = extra_exp
    _PSUM_SPLIT = psum_split
    _HALF_EXP = half_exp
    nc = bacc.Bacc("TRN2", target_bir_lowering=False, debug=False,
                   enable_asserts=False, num_devices=8)

    xT = nc.dram_tensor("xT", [D, T], F16, kind="ExternalInput").ap()
    wqkT = nc.dram_tensor("wqkT", [D, 2 * DH], F16, kind="ExternalInput").ap()
    wvT = nc.dram_tensor("wvT", [D, DH], F16, kind="ExternalInput").ap()
    bqk = nc.dram_tensor("bqk", [P, NKT], F32, kind="ExternalInput").ap()
    bv = nc.dram_tensor("bv", [1, DH], F16, kind="ExternalInput").ap()
    projT = nc.dram_tensor("projT", [DH, D], F16, kind="ExternalInput").ap()
    out = nc.dram_tensor("out", [T, D], F32, kind="ExternalOutput").ap()

    with tile.TileContext(nc) as tc:
        with ExitStack() as ctx:
            _emit(nc, tc, ctx, xT, wqkT, wvT, bqk, bv, projT, out, reps)
    nc.compile()
    return nc


def _emit(nc, tc, ctx, xT, wqkT, wvT, bqk, bv, projT, out, reps=1):
    pers = ctx.enter_context(tc.tile_pool(name="pers", bufs=1))
    apool = ctx.enter_context(tc.tile_pool(name="apool", bufs=22))
    rpool = ctx.enter_context(tc.tile_pool(name="rpool", bufs=2))
    opool = ctx.enter_context(tc.tile_pool(name="opool", bufs=3))
    pspool = ctx.enter_context(tc.tile_pool(name="pspool", bufs=1, space="PSUM"))

    # ---- constant / weight / input loads ----
    bqk_sb = pers.tile([P, NKT], F32, name="bqk_sb")
    nc.sync.dma_start(bqk_sb[:], bqk[:])
    bv_sb = pers.tile([1, DH], F16, name="bv_sb")
    nc.sync.dma_start(bv_sb[:], bv[:])
    bvB = pers.tile([P, DH], F16, name="bvB")
    nc.gpsimd.partition_broadcast(bvB[:], bv_sb[:])
    # dummy exp: pulls the one-time ACT exp-table load (~2.7us) into the DMA
    # head so the first real S-tile exp doesn't pay it
    warm = pers.tile([P, NKT], F16, name="warm")
    nc.scalar.activation(warm[:], bqk_sb[:],
                         mybir.ActivationFunctionType.Exp, scale=0.001)

    wqk_sb = []
    x_sb = []
    wv_sb = []
    for ki in range(NKT):
        w = pers.tile([P, 2 * DH], F16, name=f"wqk_sb{ki}")
        nc.sync.dma_start(w[:], wqkT[ki * P:(ki + 1) * P, :])
        wqk_sb.append(w)
        xx = pers.tile([P, T], F16, name=f"x_sb{ki}")
        nc.sync.dma_start(xx[:], xT[ki * P:(ki + 1) * P, :])
        x_sb.append(xx)
    for ki in range(NKT):
        w = pers.tile([P, DH], F16, name=f"wv_sb{ki}")
        nc.sync.dma_start(w[:], wvT[ki * P:(ki + 1) * P, :])
        wv_sb.append(w)
    projT_sb = []
    for ki in range(DH // P):
        w = pers.tile([P, D], F16, name=f"projT_sb{ki}")
        nc.sync.dma_start(w[:], projT[ki * P:(ki + 1) * P, :])
        projT_sb.append(w)

    qk_sb = [pers.tile([P, T], F16, name=f"qk_sb{mi}") for mi in range(NKT)]
    vbuf = [pers.tile([P, NH, DK + 1], F16, name=f"vbuf{ti}") for ti in range(NTT)]
    zt_sb = [pers.tile([P, T], F16, name=f"zt_sb{ki}") for ki in range(DH // P)]

    for ti in range(NTT):
        nc.vector.memset(vbuf[ti][:, :, DK:DK + 1], 1.0)

    for rep in range(reps):
        _emit_compute(nc, tc, pers, apool, rpool, opool, pspool,
                      wqk_sb, x_sb, wv_sb, projT_sb, qk_sb, vbuf, zt_sb,
                      bqk_sb, bvB, out, rep)


def _emit_compute(nc, tc, pers, apool, rpool, opool, pspool,
                  wqk_sb, x_sb, wv_sb, projT_sb, qk_sb, vbuf, zt_sb,
                  bqk_sb, bvB, out, rep=0):

    def emit_qk(mi):
        for tcn in range(NQC):
            ps = pspool.tile([P, 512], F32, tag="m" if _PSUM_SPLIT else "z",
                             bufs=2, name=f"ps_qk{mi}_{tcn}")
            for ki in range(NKT):
                nc.tensor.matmul(
                    ps[:],
                    wqk_sb[ki][:, mi * P:(mi + 1) * P],
                    x_sb[ki][:, tcn * 512:(tcn + 1) * 512],
                    start=(ki == 0), stop=(ki == NKT - 1),
                )
            nc.vector.tensor_scalar_add(
                qk_sb[mi][:, tcn * 512:(tcn + 1) * 512], ps[:],
                bqk_sb[:, mi:mi + 1],
            )

    def emit_v(ti):
        ps = pspool.tile([P, 512], F32, tag="m" if _PSUM_SPLIT else "z",
                         bufs=2, name=f"ps_v{ti}")
        for ki in range(NKT):
            nc.tensor.matmul(
                ps[:],
                x_sb[ki][:, ti * P:(ti + 1) * P],
                wv_sb[ki][:],
                start=(ki == 0), stop=(ki == NKT - 1),
            )
        nc.vector.tensor_add(
            vbuf[ti][:, :, 0:DK],
            ps.rearrange("p (h j) -> p h j", h=NH),
            bvB.rearrange("p (h j) -> p h j", h=NH),
        )

    # Head pairs are row-packed on the PE: even head uses array rows 0-63,
    # odd head rows 64-127; their S matmuls run concurrently. One psum tile
    # [128, 2, 512] holds both heads' S^T chunk for a kt tile; one exp
    # instruction covers both.
    def emit_attn(qc, pr):
        qcs = slice(qc * 512, (qc + 1) * 512)
        lhe, lho = 2 * pr, 2 * pr + 1
        qtile = qk_sb[pr]
        ktile = qk_sb[4 + pr]
        q_e = qtile[0:64, qcs]
        q_o = qtile[64:128, qcs]

        a_tiles = []
        for kt in range(NTT):
            kts = slice(kt * P, (kt + 1) * P)
            ps = pspool.tile([P, 2, 512], F32, tag="s",
                             bufs=2 if _PSUM_SPLIT else 3,
                             name=f"ps_s{qc}_{pr}_{kt}")
            nc.tensor.matmul(ps[:, 0, :], ktile[0:64, kts], q_e,
                             start=True, stop=True, tile_position=(0, 0))
            nc.tensor.matmul(ps[:, 1, :], ktile[64:128, kts], q_o,
                             start=True, stop=True, tile_position=(64, 0))
            a = apool.tile([P, 2, 512], F16, tag="A",
                           name=f"a_{qc}_{pr}_{kt}")
            if _HALF_EXP:
                # timing diagnostic: half the ACT work, identical PE/DVE work
                nc.scalar.activation(a[:, 0, :], ps[:, 0, :],
                                     mybir.ActivationFunctionType.Exp,
                                     scale=SCALE)
            else:
                nc.scalar.activation(a, ps[:],
                                     mybir.ActivationFunctionType.Exp,
                                     scale=SCALE)
            if _EXTRA_EXP:
                a2 = apool.tile([P, 2, 512], F16, tag="Ascr", bufs=2,
                                name=f"a2_{qc}_{pr}_{kt}")
                nc.scalar.activation(a2, ps[:],
                                     mybir.ActivationFunctionType.Exp,
                                     scale=SCALE)
            a_tiles.append(a)

        # z accumulations for the two heads interleave kt-wise so each A
        # tile is released as soon as both heads consumed it.
        psz_e = pspool.tile([P, 512], F32, tag="z", bufs=2,
                            name=f"ps_z{qc}_{lhe}")
        psz_o = pspool.tile([P, 512], F32, tag="z", bufs=2,
                            name=f"ps_z{qc}_{lho}")
        for kt in range(NTT):
            for psz, idx, lh in ((psz_e, 0, lhe), (psz_o, 1, lho)):
                nc.tensor.matmul(
                    psz[0:DK + 1, :],
                    vbuf[kt][:, lh, :],
                    a_tiles[kt][:, 0 if _HALF_EXP else idx, :],
                    start=(kt == 0), stop=(kt == NTT - 1),
                )
        for psz, idx, lh in ((psz_e, 0, lhe), (psz_o, 1, lho)):
            row = 64 * idx
            recip = rpool.tile([1, 512], F32, tag="rc", name=f"rc_{qc}_{lh}")
            nc.vector.reciprocal(recip[:], psz[DK:DK + 1, :])
            recipB = rpool.tile([64, 512], F32, tag="rb", name=f"rb_{qc}_{lh}")
            nc.gpsimd.partition_broadcast(recipB[:], recip[:])
            nc.vector.tensor_mul(
                zt_sb[pr][row:row + 64, qcs],
                psz[0:DK, :],
                recipB[:],
            )

    def emit_proj(ti):
        ot = opool.tile([P, D], F32, tag="ot", name=f"ot{ti}")
        for ocn in range(2):
            ps = pspool.tile([P, 512], F32, tag="m" if _PSUM_SPLIT else "z",
                             bufs=2, name=f"ps_o{ti}_{ocn}")
            for ki in range(DH // P):
                nc.tensor.matmul(
                    ps[:],
                    zt_sb[ki][:, ti * P:(ti + 1) * P],
                    projT_sb[ki][:, ocn * 512:(ocn + 1) * 512],
                    start=(ki == 0), stop=(ki == DH // P - 1),
                )
            nc.vector.tensor_copy(ot[:, ocn * 512:(ocn + 1) * 512], ps[:])
        nc.sync.dma_start(out[ti * P:(ti + 1) * P, :], ot[:])

    # Emission order: q,k feature tiles (head pairs early), v, then qc-outer
    # attention with the qc's token projections right after each qc completes.
    for mi in (0, 4, 1, 5, 2, 6, 3, 7):
        emit_qk(mi)
    for ti in range(NTT):
        emit_v(ti)
    for qc in range(NQC):
        for pr in range(NH // 2):
            emit_attn(qc, pr)
        for ti in range(qc * NTT // NQC, (qc + 1) * NTT // NQC):
            emit_proj(ti)


_NC_CACHE = None


def _get_program():
    global _NC_CACHE
    if _NC_CACHE is None:
        _NC_CACHE = build_program()
    return _NC_CACHE


def shard_inputs(x, Win_w, Win_b, proj_w, proj_b):
    """Build the 8 per-core input maps (host-side numpy)."""
    in_maps = []
    for c in range(8):
        b, g = divmod(c, 2)
        qs = slice(g * DH, (g + 1) * DH)
        ks = slice(D + g * DH, D + (g + 1) * DH)
        vs = slice(2 * D + g * DH, 2 * D + (g + 1) * DH)
        wqk = np.concatenate([Win_w[qs], Win_w[ks]], axis=0)      # [1024, 1024]
        bqk_v = np.concatenate([Win_b[qs], Win_b[ks]])            # [1024]
        in_maps.append({
            "xT": np.ascontiguousarray(x[b].T.astype(np.float16)),
            "wqkT": np.ascontiguousarray(wqk.T.astype(np.float16)),
            "wvT": np.ascontiguousarray(Win_w[vs].T.astype(np.float16)),
            "bqk": np.ascontiguousarray(
                bqk_v.reshape(NKT, P).T.astype(np.float32)),
            "bv": Win_b[vs].astype(np.float16).reshape(1, DH),
            "projT": np.ascontiguousarray(
                proj_w[:, g * DH:(g + 1) * DH].T.astype(np.float16)),
        })
    return in_maps


def combine_outputs(results, proj_b):
    out = np.empty((B, L, D), dtype=np.float32)
    pb = proj_b.astype(np.float32)
    for b in range(B):
        out[b] = results[2 * b]["out"] + results[2 * b + 1]["out"] + pb
    return out


def kernel(x, Win_w, Win_b, proj_w, proj_b):
    x = np.asarray(x, dtype=np.float32)
    Win_w = np.asarray(Win_w, dtype=np.float32)
    Win_b = np.asarray(Win_b, dtype=np.float32)
    proj_w = np.asarray(proj_w, dtype=np.float32)
    proj_b = np.asarray(proj_b, dtype=np.float32)

    nc = _get_program()
    in_maps = shard_inputs(x, Win_w, Win_b, proj_w, proj_b)
    res = run_bass_kernel_spmd(nc, in_maps, core_ids=list(range(8)))
    return combine_outputs(res.results, proj_b)
